# revision 38
# baseline (speedup 1.0000x reference)
"""Trainium2 Bass kernel for a heterogeneous GraphConv layer (3 relations).

out = concat([leaky(GC(inst_feat, W_inst, in_*)),     # -> node   (10000)
              leaky(GC(node_feat, W_node, ni_*)),     # -> inst   (100000)
              leaky(GC(svc_feat,  W_svc,  sc_*))])    # -> svc    (20000)

GC(f, W, src, dst) = rsqrt(deg_d) * segsum_dst((rsqrt(deg_s)*f)[src]) @ W + b
(aggregation commutes with the dense @W, so we gather *raw scaled features*
and apply W once per destination tile).

Strategy: destination-sharded across 8 NeuronCores. Per core and relation the
edges (sorted by dst) are grouped into 128-edge blocks per 128-dst tile.
Device work per block: dma_gather 128 source rows from DRAM -> [128e,128f],
DVE builds a one-hot S^T[e,dst]=(iota==dst_local) tile, PE accumulates
aggT[f,dst] += G.T @ S^T in PSUM.  Per dst tile: PSUM out = u (x) b (rank-1
K=1 matmul preloading bias/rsqrt_deg_d) + aggT.T @ W, then one ScalarE
Lrelu(out * rsqrt_deg_d) and a DMA to the output rows.
"""

import os as _os

import numpy as np

SVC_N, INST_N, NODE_N, HID = 20000, 100000, 10000, 128
NCORES = 8
BLK = 128           # edges per one-hot matmul block
# blocks per dma_gather instruction. NOTE: needs single_packet=False — with
# single_packet=True the whole stream coalesces into one DMA packet, which
# caps at 64 descriptors/engine (num_idxs <= 1024); beyond that the exec
# unit faults (NRT_EXEC_UNIT_UNRECOVERABLE).
CHUNK = int(_os.environ.get("GNN_CHUNK", "32"))
# one-hot builder batching: 1 = tensor_scalar per block; >1 = one DVE
# tensor_tensor(is_equal) per EQG consecutive blocks (dl broadcast stride-0)
EQG = int(_os.environ.get("GNN_EQG", "1"))
# gather/one-hot dtype: fp16 (default) has a 10-bit mantissa (~4x better
# than bf16), runs the PE at 1 cyc/row like bf16, and halves gather bytes
# vs fp32. GNN_BF16=1 kept for A/B.
GDT = _os.environ.get("GNN_DT", "fp16")
if int(_os.environ.get("GNN_BF16", "0")):
    GDT = "bf16"
# split mode: tables hold [hi|lo] half-precision pairs (512B rows); two
# accumulating matmuls per block recover ~fp32 precision at 2x PE cost
USE_SPLIT = bool(int(_os.environ.get("GNN_SPLIT", "0")))
if USE_SPLIT and GDT == "fp32":
    GDT = "fp16"
USE_BF16 = GDT != "fp32"  # legacy name: True when gather dtype is 16-bit
ACT_MODE = "lrelu"  # "lrelu" (HW leaky relu) | "relu" (sim debug)

_cache = {}


def _cdiv(a, b):
    return (a + b - 1) // b


def _rup(a, b):
    return _cdiv(a, b) * b


def _gdt_np():
    if GDT == "fp32":
        return np.dtype(np.float32)
    if GDT == "fp16":
        return np.dtype(np.float16)
    import ml_dtypes
    return np.dtype(ml_dtypes.bfloat16)


def _prep_relation(src, dst, n_src, n_dst, feat, compact):
    """Host-side sharding/packing for one relation."""
    src = np.asarray(src, np.int64)
    dst = np.asarray(dst, np.int64)
    deg_s = np.maximum(np.bincount(src, minlength=n_src), 1).astype(np.float64)
    deg_d = np.maximum(np.bincount(dst, minlength=n_dst), 1).astype(np.float64)
    rs_s = (1.0 / np.sqrt(deg_s)).astype(np.float32)
    rs_d = (1.0 / np.sqrt(deg_d)).astype(np.float32)
    u_d = np.sqrt(deg_d).astype(np.float32)  # ~= 1/rs_d

    feat_s = (np.asarray(feat, np.float32) * rs_s[:, None]).astype(np.float32)

    D = _rup(_cdiv(n_dst, NCORES), 128)  # dst rows per core (padded)
    ntiles = D // 128

    cores = []
    counts = np.zeros((NCORES, ntiles), np.int64)
    for c in range(NCORES):
        lo, hi = c * D, (c + 1) * D
        m = (dst >= lo) & (dst < hi)
        es, ed = src[m], dst[m] - lo
        order = np.argsort(ed, kind="stable")
        es, ed = es[order], ed[order]
        if compact:
            uniq, inv = np.unique(es, return_inverse=True)
            gidx = inv.astype(np.int64)
            table = feat_s[uniq]
        else:
            gidx = es
            table = None
        tile_of = ed >> 7
        counts[c] = np.bincount(tile_of, minlength=ntiles)
        cores.append(dict(gidx=gidx, dloc=(ed & 127).astype(np.float32),
                          table=table, tile_counts=counts[c]))

    # blocks per tile: max over cores, >=1 so PSUM always gets a start matmul
    btile = np.maximum(_cdiv(np.max(counts, axis=0), BLK), 1).astype(np.int64)
    nblk = int(btile.sum())
    nblk_pad = _rup(nblk, CHUNK)

    for c in range(NCORES):
        d = cores[c]
        g = np.zeros(nblk_pad * BLK, np.int64)
        dl = np.full(nblk_pad * BLK, -1.0, np.float32)
        pos = 0  # edge cursor in sorted arrays
        off = 0  # block-slot cursor
        for t in range(ntiles):
            n = int(d["tile_counts"][t])
            g[off:off + n] = d["gidx"][pos:pos + n]
            dl[off:off + n] = d["dloc"][pos:pos + n]
            pos += n
            off += int(btile[t]) * BLK
        # relation-tail pad slots: idx -1 -> trailing negatives are skipped
        # by the gather engine (num_idxs_reg trimmed per chunk device-side)
        g[nblk * BLK:] = -1
        d["gflat"], d["dlflat"] = g, dl
        del d["gidx"], d["dloc"]

    return dict(cores=cores, btile=btile, nblk=nblk, nblk_pad=nblk_pad,
                ntiles=ntiles, D=D, feat_s=feat_s, rs_d=rs_d, u_d=u_d,
                n_dst=n_dst, compact=compact)


def _build_host_data(inputs):
    rels = [
        # order matters: output rows are [node_out, inst_out, svc_out]
        _prep_relation(inputs["in_src"], inputs["in_dst"], INST_N, NODE_N,
                       inputs["instance_feat"], compact=True),
        _prep_relation(inputs["ni_src"], inputs["ni_dst"], NODE_N, INST_N,
                       inputs["node_feat"], compact=False),
        _prep_relation(inputs["sc_src"], inputs["sc_dst"], SVC_N, SVC_N,
                       inputs["svc_feat"], compact=False),
    ]
    Ws = [inputs["W_inst"], inputs["W_node"], inputs["W_svc"]]
    bs = [inputs["b_inst"], inputs["b_node"], inputs["b_svc"]]

    gdt = _gdt_np()

    umax = _rup(max(len(rels[0]["cores"][c]["table"]) for c in range(NCORES)), 16)
    nblk_tot = sum(r["nblk_pad"] for r in rels)
    nidx_tot = nblk_tot * BLK
    ntile_tot = sum(r["ntiles"] for r in rels)

    W_cat = np.concatenate([np.asarray(w, np.float32) for w in Ws], axis=1)
    b_row = np.concatenate([np.asarray(b, np.float32) for b in bs])[None, :]
    iota = np.tile(np.arange(128, dtype=np.float32), (128, max(1, EQG))).astype(gdt)

    in_maps = []
    for c in range(NCORES):
        gidx = np.concatenate([r["cores"][c]["gflat"] for r in rels])
        dl = np.concatenate([r["cores"][c]["dlflat"] for r in rels])
        assert gidx.max() < 32768, "gather idx must fit int16"
        # gather layout: idx i -> partition i%16, col i//16, replicated x8
        idx16 = np.ascontiguousarray(gidx.astype(np.int16).reshape(-1, 16).T)
        idx_sb = np.tile(idx16, (8, 1))                          # [128, nidx/16]
        dl_sb = np.ascontiguousarray(
            dl.reshape(nblk_tot, BLK).T).astype(np.float32)      # [128, nblk]

        rs_sb = np.zeros((128, ntile_tot), np.float32)
        u_sb = np.zeros((1, ntile_tot * 128), np.float32)
        t0 = 0
        for r in rels:
            lo = c * r["D"]
            val_rs = np.zeros(r["D"], np.float32)
            val_u = np.zeros(r["D"], np.float32)
            n = max(0, min(r["D"], r["n_dst"] - lo))
            if n > 0:
                val_rs[:n] = r["rs_d"][lo:lo + n]
                val_u[:n] = r["u_d"][lo:lo + n]
            rs_sb[:, t0:t0 + r["ntiles"]] = val_rs.reshape(r["ntiles"], 128).T
            u_sb[0, t0 * 128:(t0 + r["ntiles"]) * 128] = val_u
            t0 += r["ntiles"]

        tbl_in = np.zeros((umax, HID), np.float32)
        t = rels[0]["cores"][c]["table"]
        tbl_in[:len(t)] = t

        def _tbl(x):
            hi = x.astype(gdt)
            if not USE_SPLIT:
                return np.ascontiguousarray(hi)
            lo = (x - hi.astype(np.float32)).astype(gdt)
            return np.ascontiguousarray(np.concatenate([hi, lo], axis=1))

        in_maps.append({
            "tbl_in": _tbl(tbl_in),
            "tbl_ni": _tbl(rels[1]["feat_s"]),
            "tbl_sc": _tbl(rels[2]["feat_s"]),
            "idx_sb": np.ascontiguousarray(idx_sb),
            "dl_sb": dl_sb,
            "rs_sb": rs_sb,
            "u_sb": u_sb,
            "W_cat": np.ascontiguousarray(W_cat),
            "b_row": np.ascontiguousarray(b_row),
            "iota": np.ascontiguousarray(iota),
        })

    meta = dict(
        umax=umax, nblk_tot=nblk_tot, nidx_tot=nidx_tot, ntile_tot=ntile_tot,
        btiles=[r["btile"].tolist() for r in rels],
        ntiles=[r["ntiles"] for r in rels],
        Ds=[r["D"] for r in rels],
        n_dsts=[r["n_dst"] for r in rels],
    )
    return meta, in_maps


def _build_program(meta):
    import os
    from contextlib import ExitStack

    import concourse.bacc as bacc
    import concourse.mybir as mybir
    import concourse.tile as tile

    dbg_max_tiles = int(os.environ.get("GNN_MAX_TILES", "0"))  # 0 = all
    dbg_skip_gather = bool(int(os.environ.get("GNN_SKIP_GATHER", "0")))
    assert CHUNK % EQG == 0, "eq-groups must align with gather chunks"
    OUT_GRP = 4  # dst tiles batched per epilogue staging buffer / out DMA

    gdt = {"fp32": mybir.dt.float32, "fp16": mybir.dt.float16,
           "bf16": mybir.dt.bfloat16}[GDT]
    f32 = mybir.dt.float32
    AF = mybir.ActivationFunctionType
    act_fn = AF.Lrelu if ACT_MODE == "lrelu" else AF.Relu

    nblk_tot, nidx_tot, ntile_tot = meta["nblk_tot"], meta["nidx_tot"], meta["ntile_tot"]
    umax = meta["umax"]

    nc = bacc.Bacc("TRN2", target_bir_lowering=False, debug=False,
                   enable_asserts=False, num_devices=NCORES)

    TW = 2 * HID if USE_SPLIT else HID  # table row width
    tbl_d = [
        nc.dram_tensor("tbl_in", [umax, TW], gdt, kind="ExternalInput"),
        nc.dram_tensor("tbl_ni", [NODE_N, TW], gdt, kind="ExternalInput"),
        nc.dram_tensor("tbl_sc", [SVC_N, TW], gdt, kind="ExternalInput"),
    ]
    idx_d = nc.dram_tensor("idx_sb", [128, nidx_tot // 16], mybir.dt.int16,
                           kind="ExternalInput")
    dl_d = nc.dram_tensor("dl_sb", [128, nblk_tot], f32, kind="ExternalInput")
    rs_d = nc.dram_tensor("rs_sb", [128, ntile_tot], f32, kind="ExternalInput")
    u_d = nc.dram_tensor("u_sb", [1, ntile_tot * 128], f32, kind="ExternalInput")
    W_d = nc.dram_tensor("W_cat", [128, 3 * HID], f32, kind="ExternalInput")
    b_d = nc.dram_tensor("b_row", [1, 3 * HID], f32, kind="ExternalInput")
    iota_d = nc.dram_tensor("iota", [128, EQG * 128], gdt, kind="ExternalInput")

    out_d = [
        nc.dram_tensor("out_node", [meta["Ds"][0], HID], f32, kind="ExternalOutput"),
        nc.dram_tensor("out_inst", [meta["Ds"][1], HID], f32, kind="ExternalOutput"),
        nc.dram_tensor("out_svc", [meta["Ds"][2], HID], f32, kind="ExternalOutput"),
    ]

    with tile.TileContext(nc) as tc:
        with (
            tc.tile_pool(name="const", bufs=1) as const,
            tc.tile_pool(name="g", bufs=4) as gpool,
            tc.tile_pool(name="st", bufs=8) as stpool,
            tc.tile_pool(name="evac", bufs=4) as evac,
            tc.tile_pool(name="osb", bufs=6) as opool,
            tc.tile_pool(name="psA", bufs=4, space="PSUM") as psA,
            tc.tile_pool(name="psO", bufs=3, space="PSUM") as psO,
        ):
            iota_t = const.tile([128, EQG * 128], gdt)
            nc.sync.dma_start(iota_t[:], iota_d.ap())
            W_t = const.tile([128, 3 * HID], f32)
            nc.sync.dma_start(W_t[:], W_d.ap())
            b_t = const.tile([1, 3 * HID], f32)
            nc.sync.dma_start(b_t[:], b_d.ap())
            u_t = const.tile([1, ntile_tot * 128], f32)
            nc.sync.dma_start(u_t[:], u_d.ap())
            rs_t = const.tile([128, ntile_tot], f32)
            nc.sync.dma_start(rs_t[:], rs_d.ap())
            dl_t = const.tile([128, nblk_tot], f32)
            nc.sync.dma_start(dl_t[:], dl_d.ap())
            idx_t = const.tile([128, nidx_tot // 16], mybir.dt.int16)
            c0 = min(4 * CHUNK * BLK // 16, nidx_tot // 16)
            nc.sync.dma_start(idx_t[:, :c0], idx_d.ap()[:, :c0])
            if c0 < nidx_tot // 16:
                nc.sync.dma_start(idx_t[:, c0:], idx_d.ap()[:, c0:])

            g_tiles = {}   # chunk -> gather tile
            st_tiles = {}  # eq-group -> one-hot tile [128, EQG, 128]

            def issue_st(gi):
                g0 = gi * EQG
                gsz = min(EQG, nblk_tot - g0)
                st = stpool.tile([128, EQG, 128], gdt, tag="st")
                if EQG == 1:
                    nc.vector.tensor_scalar(
                        st[:, 0, :], iota_t[:, 0:128], dl_t[:, g0:g0 + 1], None,
                        mybir.AluOpType.is_equal)
                else:
                    nc.vector.tensor_tensor(
                        st[:, :gsz, :],
                        iota_t[:, :gsz * 128].rearrange("p (g k) -> p g k", k=128),
                        dl_t[:, g0:g0 + gsz].broadcast_to([128, gsz, 128]),
                        mybir.AluOpType.is_equal)
                st_tiles[gi] = st

            def issue_gather(ci, rel, rel_blk0, rel_nblk):
                gt = gpool.tile([128, CHUNK, TW], gdt, tag="g")
                nidx = CHUNK * BLK
                # trailing -1 idxs (relation-tail pads) are skipped; trim reg
                local0 = ci * CHUNK - rel_blk0
                real_blocks = max(0, min(CHUNK, rel_nblk - local0))
                if dbg_skip_gather:
                    nc.vector.memset(gt[:], 1.0)
                else:
                    nc.gpsimd.dma_gather(
                        out_ap=gt[:],
                        in_ap=tbl_d[rel].ap(),
                        idxs_ap=idx_t[:, ci * (nidx // 16):(ci + 1) * (nidx // 16)],
                        num_idxs=nidx,
                        num_idxs_reg=max(BLK, real_blocks * BLK),
                        elem_size=TW,
                        single_packet=False,
                    )
                g_tiles[ci] = gt

            blk = 0   # global block cursor
            tglob = 0  # global tile cursor
            for rel in range(3):
                ntiles = meta["ntiles"][rel]
                btile = meta["btiles"][rel]
                rel_base = blk
                rel_nblk = sum(btile)
                for t in range(ntiles):
                    if dbg_max_tiles and t >= dbg_max_tiles:
                        break
                    nb = btile[t]
                    agg = psA.tile([128, 128], f32, tag="agg")
                    for b in range(nb):
                        ci, cj = divmod(blk, CHUNK)
                        if cj == 0:
                            issue_gather(ci, rel, rel_base, rel_nblk)
                        gi, gj = divmod(blk, EQG)
                        if gj == 0:
                            issue_st(gi)
                        for part in range(2 if USE_SPLIT else 1):
                            nc.tensor.matmul(
                                agg[:],
                                g_tiles[ci][:, cj, part * HID:(part + 1) * HID],
                                st_tiles[gi][:, gj, :],
                                start=(b == 0 and part == 0),
                                stop=(b == nb - 1 and part == (1 if USE_SPLIT else 0)))
                        blk += 1
                    # pad blocks at relation tail are gathered but never used
                    aggsb = evac.tile([128, 128], f32, tag="evac")
                    nc.scalar.copy(aggsb[:], agg[:])
                    po = psO.tile([128, 128], f32, tag="po")
                    nc.tensor.matmul(
                        po[:], u_t[:, tglob * 128:(tglob + 1) * 128],
                        b_t[:, rel * HID:(rel + 1) * HID],
                        start=True, stop=False, skip_group_check=True)
                    nc.tensor.matmul(
                        po[:], aggsb[:], W_t[:, rel * HID:(rel + 1) * HID],
                        start=False, stop=True, skip_group_check=True)
                    oj = t % OUT_GRP
                    if oj == 0:
                        osb = opool.tile([128, OUT_GRP, 128], f32, tag="osb")
                        osb_t0 = t
                    nc.scalar.activation(
                        osb[:, oj, :], po[:], act_fn,
                        bias=0.0, scale=rs_t[:, tglob:tglob + 1], alpha=0.01)
                    if oj == OUT_GRP - 1 or t == ntiles - 1:
                        cnt = t - osb_t0 + 1
                        dst = out_d[rel].ap()[osb_t0 * 128:(t + 1) * 128, :]
                        nc.sync.dma_start(
                            dst.rearrange("(j p) k -> p j k", p=128),
                            osb[:, :cnt, :])
                    tglob += 1
                # advance block cursor past relation tail padding
                rel_end_pad = (-blk) % CHUNK
                blk += rel_end_pad

    nc.compile()
    return nc


def _run(nc, in_maps, trace=False, **kw):
    from concourse import bass_utils
    res = bass_utils.run_bass_kernel_spmd(
        nc, in_maps, core_ids=list(range(NCORES)), trace=trace, **kw)
    return res


def _assemble(results, meta):
    out = np.empty((NODE_N + INST_N + SVC_N, HID), np.float32)
    offs = [0, NODE_N, NODE_N + INST_N]
    names = ["out_node", "out_inst", "out_svc"]
    for rel in range(3):
        D, n_dst = meta["Ds"][rel], meta["n_dsts"][rel]
        for c in range(NCORES):
            lo = c * D
            n = max(0, min(D, n_dst - lo))
            if n > 0:
                out[offs[rel] + lo: offs[rel] + lo + n] = \
                    results[c][names[rel]][:n]
    return out


def kernel(**inputs):
    key = "prog"
    sig = tuple(np.asarray(inputs[k]).tobytes()[:64] for k in
                ("sc_src", "sc_dst", "in_src", "in_dst", "ni_src", "ni_dst"))
    if key in _cache and _cache[key][0] == sig:
        _, nc, meta = _cache[key]
        _, in_maps = _build_host_data(inputs)
    else:
        meta, in_maps = _build_host_data(inputs)
        nc = _build_program(meta)
        _cache[key] = (sig, nc, meta)
    res = _run(nc, in_maps)
    return _assemble(res.results, meta)
